# revision 5
# baseline (speedup 1.0000x reference)
"""Trainium2 Bass kernel for a dense transformer encoder layer.

Problem shapes (hardcoded): B=16, L=1024, D=256, H=4 heads (E=64), F=512 (two
gelu FFN matmuls), fp32 I/O.  Sharding: pure data-parallel over batch across 8
NeuronCores (2 batch elements per core, no collectives).

Per-core layout strategy:
  - x^T, Q^T, K^T kept transposed [D, T] (bf16) so attention scores
    S = q^T.T @ k^T come out natural [l, s]; two heads run concurrently on the
    PE array via row tiling (K=64 at partition offsets 0/64).
  - attn bias is DMA'd [128, 1024] tiles; added to S either by a fp32
    identity-matmul accumulated into the same PSUM group (PE) or by DVE,
    split ~50/50 to balance engines.
  - A = exp(logits) written bf16, transposed 128x128 on the PE (bf16 identity),
    copied back PSUM->SBUF alternating DVE/ACT.
  - A@V uses V in natural layout [s, e] augmented with a ones column (M=65) so
    the softmax denominator falls out of row 64 of the PSUM; ctx^T is then
    normalized with a gpsimd-broadcast reciprocal row.
  - LN rstd = exp(-0.5*ln(var+eps)) keeps ScalarE inside the ln/exp table set
    (avoids table thrash with softmax exp); FFN gelus run after via dep chain.
"""

import functools

import numpy as np

B, L, D, H, E, F = 16, 1024, 256, 4, 64, 512
NCORES = 8
BPC = B // NCORES          # batches per core = 2
T = BPC * L                # tokens per core = 2048
P = 128
KC = D // P                # 2 d-chunks
FC = F // P                # 4 f-chunks
TC = T // P                # 16 token chunks
NT4 = T // 512             # 4 token 512-chunks
SC8 = L // P               # 8 seq chunks per batch
EPS = 1e-5
SCALE = 1.0 / np.sqrt(E)


def _emit(tc_ctx, nc, hd):
    import concourse.bass as bass
    import concourse.mybir as mybir
    from concourse.masks import make_identity

    f32 = mybir.dt.float32
    bf16 = mybir.dt.bfloat16
    ADD = mybir.AluOpType.add
    MULT = mybir.AluOpType.mult
    SUB = mybir.AluOpType.subtract
    AF = mybir.ActivationFunctionType

    tc = tc_ctx
    ctx = tc._emit_ctx  # ExitStack stored by caller

    wpool = ctx.enter_context(tc.tile_pool(name="w", bufs=1))
    xpool = ctx.enter_context(tc.tile_pool(name="x", bufs=1))
    biasp = ctx.enter_context(tc.tile_pool(name="bias", bufs=2))
    apool = ctx.enter_context(tc.tile_pool(name="a", bufs=2))
    atpool = ctx.enter_context(tc.tile_pool(name="at", bufs=1))
    small = ctx.enter_context(tc.tile_pool(name="small", bufs=2))
    ps_s = ctx.enter_context(tc.tile_pool(name="pss", bufs=2, space="PSUM"))
    ps_t = ctx.enter_context(tc.tile_pool(name="pst", bufs=2, space="PSUM"))
    ps_av = ctx.enter_context(tc.tile_pool(name="psav", bufs=1, space="PSUM"))
    ps_mm = ctx.enter_context(tc.tile_pool(name="psmm", bufs=2, space="PSUM"))
    dpool = ctx.enter_context(tc.tile_pool(name="dsc", bufs=2, space="DRAM"))

    # ---------------- weights / constants ----------------
    def rep_load(name, n):
        # replicate a [n] dram vector across 128 partitions
        t = wpool.tile([P, n], f32, tag=name)
        src = hd[name][:]
        nc.gpsimd.dma_start(
            out=t, in_=bass.AP(tensor=src.tensor, offset=src.offset,
                               ap=[[0, P]] + list(src.ap))
        )
        return t

    def wload(name, kchunks, n, tag):
        t = wpool.tile([P, kchunks, n], bf16, tag=tag)
        nc.gpsimd.dma_start(out=t, in_=hd[name][:].rearrange("(kc p) n -> p kc n", p=P))
        return t

    wq = wload("Wq", KC, D, "wq")
    wk = wload("Wk", KC, D, "wk")
    wv = wload("Wv", KC, D, "wv")
    wo = wload("Wo", KC, D, "wo")
    w1 = wload("W1", KC, F, "w1")
    w2 = wload("W2", FC, D, "w2")

    bq = wpool.tile([P, KC], f32, tag="bq")
    nc.sync.dma_start(bq, hd["bq"][:].rearrange("(mc p) -> p mc", p=P))
    nc.vector.tensor_scalar_mul(bq, bq, SCALE)
    bk = wpool.tile([P, KC], f32, tag="bk")
    nc.sync.dma_start(bk, hd["bk"][:].rearrange("(mc p) -> p mc", p=P))
    b1 = wpool.tile([P, FC], f32, tag="b1")
    nc.sync.dma_start(b1, hd["b1"][:].rearrange("(mc p) -> p mc", p=P))

    bv_rep = rep_load("bv", D)
    bo_rep = rep_load("bo", D)
    b2_rep = rep_load("b2", D)
    g1_rep = rep_load("ln1_g", D)
    be1_rep = rep_load("ln1_b", D)
    g2_rep = rep_load("ln2_g", D)
    be2_rep = rep_load("ln2_b", D)

    ident_f = wpool.tile([P, P], f32, tag="idf")
    make_identity(nc, ident_f)
    ident_b = wpool.tile([P, P], bf16, tag="idb")
    make_identity(nc, ident_b)
    eps_t = wpool.tile([P, 1], f32, tag="eps")
    nc.vector.memset(eps_t, EPS)

    # ---------------- x load + transpose ----------------
    x_sb = xpool.tile([P, TC, D], f32, tag="x")
    x_ap = hd["x"][:].flatten_outer_dims().rearrange("(t p) d -> p t d", p=P)
    nc.sync.dma_start(x_sb, x_ap)

    xT = xpool.tile([P, KC, T], bf16, tag="xT")
    for t in range(TC):
        for c in range(KC):
            pst = ps_t.tile([P, P], f32, tag="tp")
            nc.tensor.transpose(pst, x_sb[:, t, c * P:(c + 1) * P], ident_f)
            nc.scalar.copy(xT[:, c, t * P:(t + 1) * P], pst)

    # ---------------- Q^T K^T V projections ----------------
    qT = xpool.tile([P, KC, T], bf16, tag="qT")
    kT = xpool.tile([P, KC, T], bf16, tag="kT")
    for w_sb, b_sb, outT, scl in ((wq, bq, qT, SCALE), (wk, bk, kT, 1.0)):
        for mc in range(KC):
            for n4 in range(NT4):
                ps = ps_mm.tile([P, 512], f32, tag="mm")
                for kc in range(KC):
                    nc.tensor.matmul(
                        ps, w_sb[:, kc, mc * P:(mc + 1) * P],
                        xT[:, kc, n4 * 512:(n4 + 1) * 512],
                        start=(kc == 0), stop=(kc == KC - 1))
                nc.scalar.activation(
                    outT[:, mc, n4 * 512:(n4 + 1) * 512], ps, AF.Identity,
                    bias=b_sb[:, mc:mc + 1], scale=scl)

    # V natural layout with ones column: [P, TC, H, E+1]
    v_sb = xpool.tile([P, TC, H, E + 1], bf16, tag="v")
    nc.vector.memset(v_sb[:, :, :, E:E + 1], 1.0)
    for t in range(TC):
        ps = ps_mm.tile([P, 512], f32, tag="mm")
        for kc in range(KC):
            nc.tensor.matmul(ps[:, :D], xT[:, kc, t * P:(t + 1) * P],
                             wv[:, kc, :], start=(kc == 0), stop=(kc == KC - 1))
        nc.vector.tensor_tensor(
            v_sb[:, t, :, 0:E], ps[:, :D].rearrange("p (h e) -> p h e", h=H),
            bv_rep.rearrange("p (h e) -> p h e", h=H), ADD)

    # ---------------- attention ----------------
    ctxT = xpool.tile([P, KC, T], bf16, tag="ctxT")
    last_exp = [None]
    for b in range(BPC):
        for hp in range(2):
            heads = (2 * hp, 2 * hp + 1)
            at_map = {}
            for h in heads:
                at_map[h] = atpool.tile([P, SC8, L], bf16, tag=f"at{h % 2}", name=f"at{h % 2}")
            for lc in range(SC8):
                bt = {}
                for h in heads:
                    bt[h] = biasp.tile([P, L], f32, tag=f"b{h % 2}", name=f"bt{h % 2}")
                    nc.sync.dma_start(
                        bt[h], hd["attn_bias"][b, h, lc * P:(lc + 1) * P, :])
                a_t = {h: apool.tile([P, L], bf16, tag=f"a{h % 2}", name=f"a{h % 2}") for h in heads}
                for si in range(2):
                    for h in heads:
                        po = (h % 2) * 64
                        ps = ps_s.tile([P, 512], f32, tag="s")
                        qh = qT[po:po + 64, hp, b * L + lc * P: b * L + (lc + 1) * P]
                        kh = kT[po:po + 64, hp, b * L + si * 512: b * L + (si + 1) * 512]
                        pe_bias = (si == 0)
                        nc.tensor.matmul(ps, qh, kh, start=True, stop=not pe_bias)
                        if pe_bias:
                            nc.tensor.matmul(ps, ident_f,
                                             bt[h][:, si * 512:(si + 1) * 512],
                                             start=False, stop=True)
                        else:
                            nc.vector.tensor_tensor(
                                ps, ps, bt[h][:, si * 512:(si + 1) * 512], ADD)
                        e_i = nc.scalar.activation(
                            a_t[h][:, si * 512:(si + 1) * 512], ps, AF.Exp)
                        last_exp[0] = e_i
                # transpose A -> AT
                for h in heads:
                    pst = ps_t.tile([P, SC8, P], bf16, tag="tp")
                    for sc in range(SC8):
                        nc.tensor.transpose(pst[:, sc, :],
                                            a_t[h][:, sc * P:(sc + 1) * P], ident_b)
                    dst = at_map[h][:, :, lc * P:(lc + 1) * P]
                    if (lc + h) % 2 == 0:
                        nc.vector.tensor_copy(dst, pst)
                    else:
                        nc.scalar.copy(dst, pst)
            # A^T @ V with ones-trick denominator
            for h in heads:
                po = (h % 2) * 64
                for l2 in range(2):
                    psc = ps_av.tile([P, 512], f32, tag="av")
                    for sc in range(SC8):
                        nc.tensor.matmul(
                            psc[:E + 1, :], v_sb[:, b * SC8 + sc, h, :],
                            at_map[h][:, sc, l2 * 512:(l2 + 1) * 512],
                            start=(sc == 0), stop=(sc == SC8 - 1))
                    rden = small.tile([1, 512], f32, tag="rden")
                    nc.vector.reciprocal(rden, psc[E:E + 1, :])
                    rdd = dpool.tile([512], f32, tag="rdd", name="rdd")
                    nc.sync.dma_start(rdd[:], rden)
                    rdb = small.tile([64, 512], f32, tag="rdb")
                    rsrc = rdd[:]
                    nc.gpsimd.dma_start(
                        out=rdb, in_=bass.AP(tensor=rsrc.tensor, offset=rsrc.offset,
                                             ap=[[0, 64]] + list(rsrc.ap)))
                    nc.vector.tensor_tensor(
                        ctxT[po:po + 64, hp, b * L + l2 * 512: b * L + (l2 + 1) * 512],
                        psc[:E, :], rdb, MULT)

    # ---------------- O proj + residual + LN1 ----------------
    h_sb = xpool.tile([P, TC, D], f32, tag="h")

    def layernorm(y_t, g_rep, b_rep, out_ap):
        st = small.tile([P, 6], f32, tag="st")
        nc.vector.bn_stats(out=st, in_=y_t)
        mv = small.tile([P, 2], f32, tag="mv")
        nc.vector.bn_aggr(out=mv, in_=st)
        lnv = small.tile([P, 1], f32, tag="lnv")
        nc.scalar.activation(lnv, mv[:, 1:2], AF.Ln, bias=eps_t[:, 0:1])
        rstd = small.tile([P, 1], f32, tag="rstd")
        nc.scalar.activation(rstd, lnv, AF.Exp, scale=-0.5)
        h0 = small.tile([P, D], f32, tag="h0")
        nc.vector.tensor_scalar(h0, y_t, scalar1=mv[:, 0:1], scalar2=rstd[:, 0:1],
                                op0=SUB, op1=MULT)
        nc.gpsimd.tensor_tensor(h0, h0, g_rep, MULT)
        nc.gpsimd.tensor_tensor(out_ap, h0, b_rep, ADD)

    for t in range(TC):
        ps = ps_mm.tile([P, 512], f32, tag="mm")
        for kc in range(KC):
            nc.tensor.matmul(ps[:, :D], ctxT[:, kc, t * P:(t + 1) * P],
                             wo[:, kc, :], start=(kc == 0), stop=(kc == KC - 1))
        y_t = small.tile([P, D], f32, tag="y")
        nc.vector.tensor_tensor(y_t, ps[:, :D], x_sb[:, t, :], ADD)
        nc.gpsimd.tensor_tensor(y_t, y_t, bo_rep, ADD)
        layernorm(y_t, g1_rep, be1_rep, h_sb[:, t, :])

    # h transpose for FFN
    hT = xpool.tile([P, KC, T], bf16, tag="hT")
    for t in range(TC):
        for c in range(KC):
            pst = ps_t.tile([P, P], f32, tag="tp")
            nc.tensor.transpose(pst[:, :P], h_sb[:, t, c * P:(c + 1) * P], ident_f)
            nc.scalar.copy(hT[:, c, t * P:(t + 1) * P], pst[:, :P])

    # ---------------- FFN1: uT = gelu(W1^T hT + b1) ----------------
    uT = xpool.tile([P, FC, T], bf16, tag="uT")
    first_gelu = [None]
    for mc in range(FC):
        for n4 in range(NT4):
            ps = ps_mm.tile([P, 512], f32, tag="mm")
            for kc in range(KC):
                nc.tensor.matmul(ps, w1[:, kc, mc * P:(mc + 1) * P],
                                 hT[:, kc, n4 * 512:(n4 + 1) * 512],
                                 start=(kc == 0), stop=(kc == KC - 1))
            g_i = nc.scalar.activation(uT[:, mc, n4 * 512:(n4 + 1) * 512], ps,
                                       AF.Gelu, bias=b1[:, mc:mc + 1])
            if first_gelu[0] is None:
                first_gelu[0] = g_i

    # ---------------- FFN2 + residual + LN2 + store ----------------
    out_flat = hd["out"][:].flatten_outer_dims().rearrange("(t p) d -> p t d", p=P)
    for t in range(TC):
        ps = ps_mm.tile([P, 512], f32, tag="mm")
        for kc in range(FC):
            nc.tensor.matmul(ps[:, :D], uT[:, kc, t * P:(t + 1) * P],
                             w2[:, kc, :], start=(kc == 0), stop=(kc == FC - 1))
        t2 = small.tile([P, D], f32, tag="t2")
        nc.vector.tensor_tensor(t2, ps[:, :D], b2_rep, ADD)
        nc.scalar.activation(t2, t2, AF.Gelu)
        nc.vector.tensor_tensor(t2, t2, h_sb[:, t, :], ADD)
        o_t = small.tile([P, D], f32, tag="o")
        layernorm(t2, g2_rep, be2_rep, o_t)
        nc.sync.dma_start(out_flat[:, t, :], o_t)


@functools.lru_cache(maxsize=1)
def _build():
    from contextlib import ExitStack

    import concourse.bacc as bacc
    import concourse.mybir as mybir
    import concourse.tile as tile

    f32 = mybir.dt.float32
    nc = bacc.Bacc("TRN2", target_bir_lowering=False)
    hd = {}
    hd["x"] = nc.dram_tensor("x", (BPC, L, D), f32, kind="ExternalInput")
    hd["attn_bias"] = nc.dram_tensor("attn_bias", (BPC, H, L, L), f32,
                                     kind="ExternalInput")
    for nm, shp in [("Wq", (D, D)), ("bq", (D,)), ("Wk", (D, D)), ("bk", (D,)),
                    ("Wv", (D, D)), ("bv", (D,)), ("Wo", (D, D)), ("bo", (D,)),
                    ("ln1_g", (D,)), ("ln1_b", (D,)), ("W1", (D, F)),
                    ("b1", (F,)), ("W2", (F, D)), ("b2", (D,)),
                    ("ln2_g", (D,)), ("ln2_b", (D,))]:
        hd[nm] = nc.dram_tensor(nm, shp, f32, kind="ExternalInput")
    hd["out"] = nc.dram_tensor("out", (BPC, L, D), f32, kind="ExternalOutput")

    with tile.TileContext(nc) as tc:
        with ExitStack() as es:
            tc._emit_ctx = es
            _emit(tc, nc, hd)
    nc.compile()
    return nc


def kernel(**inputs):
    from concourse.bass_utils import run_bass_kernel_spmd

    nc = _build()
    in_maps = []
    for c in range(NCORES):
        m = {}
        for k, v in inputs.items():
            v = np.ascontiguousarray(v, dtype=np.float32)
            if k == "x":
                m[k] = np.ascontiguousarray(v[c * BPC:(c + 1) * BPC])
            elif k == "attn_bias":
                m[k] = np.ascontiguousarray(v[c * BPC:(c + 1) * BPC])
            else:
                m[k] = v
        in_maps.append(m)
    res = run_bass_kernel_spmd(nc, in_maps, core_ids=list(range(NCORES)))
    return np.concatenate([r["out"] for r in res.results], axis=0)
